# revision 45
# baseline (speedup 1.0000x reference)
"""DigitCaps dynamic-routing kernel for Trainium2 (8 NeuronCores, Bass/Tile).

Problem: B=256, IN_CAPS=3200, IN_DIM=8, OUT_CAPS=8, OUT_DIM=16, 3 routing
iterations.  Data-parallel over batch: 32 batches per core.

Per core:
  - u_hat ([32, 3200, 128] = 52MB fp32) never exists in DRAM.  It is
    recreated per routing iteration in SBUF (bf16) in [jm=128-partition,
    (b, i)] layout via full-K=64 matmuls over a host-prepared
    block-diagonal x operand.
  - a-pass: a^T[i, j] tiles via matmuls with u_hat jm-tiles stationary and
    a block-diagonal v ("vblk") as the 8-column moving operand; output
    lands [i-partition, j-free] so softmax over j is a free-axis op on
    128 partitions.
  - s-pass needs u_hat with i on partitions: built once per half via the
    XBAR DMA-transpose into a resident bf16 copy, then s = sum_i c*u_hat
    as 25 accumulating matmuls per batch (i-layout tile stationary,
    c[i-part, j] moving).
  - squash chains run on [batch-partition] transposed views (PE transpose)
    so per-batch scalars are per-partition scalars.
"""

import sys

if "/opt/trn_rl_repo" not in sys.path:
    sys.path.insert(0, "/opt/trn_rl_repo")

import ml_dtypes
import numpy as np

import bass_rust
import concourse.bass as bass
import concourse.mybir as mybir
import concourse.tile as tile
from concourse._compat import with_exitstack
from concourse.bass_utils import run_bass_kernel_spmd
from concourse.vector_clock import ScopedClock

# ---------------------------------------------------------------------------
# Walrus on this toolchain rejects multi-wait CTRL instructions;
# TileContext's tail drain aggregates one wait per outstanding semaphore.
# Split the waits across consecutive SP drains.
_TILE_PATCHED = False


def _drain_and_barrier_split(self, tick_clock, wait_clock):
    drain_inst = self.nc.sync.drain()
    wait_clock.add_sem_waits(
        drain_inst.ins, ScopedClock({None: tick_clock.global_clock})
    )
    mi = drain_inst.ins
    waits = list(mi.sync_info.on_wait) if mi.sync_info else []
    if len(waits) > 1:
        si = mi.sync_info
        si.on_wait = waits[:1]
        mi.sync_info = si
        for i in range(1, len(waits)):
            extra = self.nc.sync.drain().ins
            extra.sync_info = bass_rust.SyncInfo(
                on_wait=waits[i : i + 1], on_update=[]
            )
    self.nc.all_engine_barrier()
    assert self.sems is not None
    popped = self.nc._tile_sem_poison_stack.pop()
    assert popped is self._sem_poison
    self.nc.clear_and_free_semaphores(list(self.sems.allocated().values()))
    self.nc.all_engine_barrier()


def _patch_tile():
    global _TILE_PATCHED
    if not _TILE_PATCHED:
        tile.TileContext._drain_and_barrier = _drain_and_barrier_split
        _TILE_PATCHED = True


_SW_COUNT = [0]


def _split_waits(nc):
    """This walrus build allows one sync wait per instruction: hoist extra
    waits onto same-engine NoOp carriers placed just before."""
    for f in nc.m.functions:
        for blk in f.blocks:
            insts = blk.instructions
            if not any(
                inst.sync_info and len(inst.sync_info.on_wait) > 1
                for inst in insts
            ):
                continue
            new = []
            for inst in insts:
                si = inst.sync_info
                waits = list(si.on_wait) if si else []
                if len(waits) > 1:
                    for w in waits[:-1]:
                        _SW_COUNT[0] += 1
                        car = mybir.InstNoOp(
                            name=f"I-sw{_SW_COUNT[0]}", engine=inst.engine
                        )
                        car.sync_info = bass_rust.SyncInfo(
                            on_wait=[w], on_update=[]
                        )
                        new.append(car)
                    si.on_wait = waits[-1:]
                    inst.sync_info = si
                new.append(inst)
            insts[:] = new


# ---------------------------------------------------------------------------
B, I, N, J, M = 256, 3200, 8, 8, 16
JM = J * M  # 128
N_CORES = 8
B_C = B // N_CORES  # 32
T = I // 128  # 25 i-tiles

IP = 16  # i's packed per K-chunk (K = IP*N = 128, uniform row group)
H = I // IP  # 200
CH_T = 128 // IP  # 8 creation chunks per 128-i tile

F32 = mybir.dt.float32
BF16 = mybir.dt.bfloat16


def _squash_chain(nc, small, ps, s_sb, ident, nb):
    """s_sb [128(jm), nb] f32 -> vT [nb, 128] f32.
    squash per capsule j: sq = sum_m s^2, v = sq*s/((1+sq)*sqrt(sq))."""
    sT_ps = ps.tile([nb, JM], F32, tag="sx")
    nc.tensor.matmul(sT_ps[:], s_sb[:], ident[:], is_transpose=True)
    sT = small.tile([nb, J, M], F32, tag="sT")
    nc.vector.tensor_copy(sT[:], sT_ps[:].rearrange("b (j m) -> b j m", m=M))
    s2 = small.tile([nb, J, M], F32, tag="s2")
    nc.vector.tensor_tensor(s2[:], sT[:], sT[:], mybir.AluOpType.mult)
    sq = small.tile([nb, J], F32, tag="sq")
    nc.vector.tensor_reduce(sq[:], s2[:], mybir.AxisListType.X, mybir.AluOpType.add)
    rt = small.tile([nb, J], F32, tag="rt")
    nc.scalar.activation(rt[:], sq[:], mybir.ActivationFunctionType.Sqrt)
    den = small.tile([nb, J], F32, tag="den")
    nc.vector.tensor_scalar_add(den[:], sq[:], 1.0)
    nc.vector.tensor_tensor(den[:], den[:], rt[:], mybir.AluOpType.mult)
    rden = small.tile([nb, J], F32, tag="rden")
    nc.vector.reciprocal(rden[:], den[:])
    scale = small.tile([nb, J], F32, tag="scale")
    nc.vector.tensor_tensor(scale[:], sq[:], rden[:], mybir.AluOpType.mult)
    vT = small.tile([nb, J, M], F32, tag="vT")
    scale_b = bass.AP(
        scale.tensor, scale[:].offset, [scale[:].ap[0], scale[:].ap[1], [0, M]]
    )
    nc.vector.tensor_tensor(vT[:], sT[:], scale_b, mybir.AluOpType.mult)
    return vT


def _vblk_from_vT(nc, small, vT, mask_rep, nb):
    """vT [nb, 128] f32 -> vblk [128(jm), nb, J] bf16 block-diagonal over j."""
    vT16 = small.tile([nb, JM], BF16, tag="vT16")
    nc.vector.tensor_copy(vT16[:], vT[:])
    vjm = small.tile([JM, nb], BF16, tag="vjm")
    nc.sync.dma_start_transpose(vjm[:], vT16[:])
    vblk = small.tile([JM, nb, J], BF16, tag="vblk_tmp")
    vjm_b = bass.AP(vjm.tensor, vjm[:].offset, [vjm[:].ap[0], vjm[:].ap[1], [0, J]])
    mask_b = bass.AP(
        mask_rep.tensor,
        mask_rep[:].offset,
        [mask_rep[:].ap[0], [0, nb], mask_rep[:].ap[1]],
    )
    nc.vector.tensor_tensor(vblk[:], vjm_b, mask_b, mybir.AluOpType.mult)
    return vblk


@with_exitstack
def build_kernel(ctx, tc, outs, ins, b_c=B_C, half=16, b_blk=2, reps=1, stage=3):
    """t-major pipeline.  b_blk is unused (kept for API compat).
    stage: 1=creation only, 2=+a+softmax, 3=full (timing ablation)."""
    nc = tc.nc
    (v_out,) = outs
    (wcr_d, xblk_d, xt_d, mask_d, ident_d) = ins
    n_half = b_c // half
    CH_T = 128 // IP  # creation chunks per 128-i tile

    const = ctx.enter_context(tc.tile_pool(name="const", bufs=1))
    res = ctx.enter_context(tc.tile_pool(name="res", bufs=1))
    scr = ctx.enter_context(tc.tile_pool(name="scr", bufs=1))
    scr2 = ctx.enter_context(tc.tile_pool(name="scr2", bufs=3))
    xs = ctx.enter_context(tc.tile_pool(name="xs", bufs=2))
    sm = ctx.enter_context(tc.tile_pool(name="sm", bufs=2))
    small = ctx.enter_context(tc.tile_pool(name="small", bufs=2))
    ps = ctx.enter_context(tc.tile_pool(name="ps", bufs=1, space="PSUM"))
    psS = ctx.enter_context(tc.tile_pool(name="psS", bufs=1, space="PSUM"))
    ps2 = ctx.enter_context(tc.tile_pool(name="ps2", bufs=2, space="PSUM"))
    ps3 = ctx.enter_context(tc.tile_pool(name="ps3", bufs=1, space="PSUM"))

    # Resident constants
    wcr = const.tile([128, H, JM], BF16)
    nc.sync.dma_start(wcr[:], wcr_d[:])
    xt = const.tile([128, H, b_c], BF16)
    nc.sync.dma_start(xt[:], xt_d[:])
    mask_rep = const.tile([JM, J], BF16)
    nc.sync.dma_start(mask_rep[:], mask_d[:])
    maskT = const.tile([J, JM], BF16)
    nc.sync.dma_start(maskT[:], mask_d[:].rearrange("a b -> b a"))
    ones8 = const.tile([J, 1], BF16)
    nc.vector.memset(ones8[:], 1.0)
    ident = const.tile([128, 128], F32)
    nc.sync.dma_start(ident[:], ident_d[:])

    for rep in range(reps):
        # ---- iteration 1 (all batches): s1 = (1/8) sum_(i,n) W x ----------
        s1_ps = ps.tile([JM, b_c], F32, tag="sx")
        for h in range(H):
            nc.tensor.matmul(
                s1_ps[:], wcr[:, h, :], xt[:, h, :],
                start=(h == 0), stop=(h == H - 1),
            )
        s_sb = small.tile([JM, b_c], F32, tag="s_sb")
        nc.vector.tensor_scalar_mul(s_sb[:], s1_ps[:], 1.0 / J)
        vT = _squash_chain(nc, small, ps, s_sb, ident, b_c)
        vblk = small.tile([JM, b_c, 2, J], BF16, tag="vblk_all")
        vb = _vblk_from_vT(nc, small, vT, mask_rep, b_c)
        nc.vector.tensor_copy(vblk[:, :, 0, :], vb[:])

        for hf in range(n_half):
            b0 = hf * half
            # u_hat i-layout resident copy for this half (t-major)
            u_res = res.tile([128, T, half, JM], BF16, tag="u_res")

            for it in (2, 3):
                nslot = it - 1
                # s accumulator: per-b [8j, jm] regions.  Zeroed explicitly;
                # the s-matmuls never set start/stop so they accumulate onto
                # the memset regardless of psum has_written/bank-zero state.
                s_ps = psS.tile([J, half, JM], F32, tag="s_ps")
                if stage >= 3:
                    nc.vector.memset(s_ps[:], 0.0)
                for t in range(T):
                    xb = xs.tile([128, CH_T, half, IP], BF16, tag="xb")
                    nc.sync.dma_start(xb[:], xblk_d[hf, t])
                    # it=2's u_t feeds the XBAR transpose whose read timing
                    # races a double-buffered rewrite on HW; keep it single-
                    # buffered.  it=3 has no XBAR, so pipeline it.
                    if it == 2:
                        u_t = scr.tile([JM, half, 128], BF16, tag="u_t")
                    else:
                        u_t = scr2.tile([JM, half, 128], BF16, tag="u_t3")
                    u_tv = u_t[:].rearrange("p b (hh i) -> p hh b i", i=IP)
                    cps = None
                    for hh in range(CH_T):
                        if hh % 2 == 0:
                            cps = ps2.tile([JM, 2, half, IP], F32, tag="cps")
                        nc.tensor.matmul(
                            cps[:, hh % 2, :, :],
                            wcr[:, t * CH_T + hh, :],
                            xb[:, hh, :, :],
                            start=True,
                            stop=True,
                        )
                        if hh % 2 == 1:
                            if (hh // 2) % 2 == 0:
                                nc.vector.tensor_copy(
                                    u_tv[:, hh - 1 : hh + 1], cps[:]
                                )
                            else:
                                nc.scalar.activation(
                                    u_tv[:, hh - 1 : hh + 1], cps[:],
                                    mybir.ActivationFunctionType.Copy,
                                )
                    if it == 2 and stage >= 3:
                        eng = nc.sync if t % 2 == 0 else nc.scalar
                        eng.dma_start_transpose(u_res[:, t, :, :], u_t[:, :, :])
                    if stage < 2:
                        continue
                    # -- a-pass: a^T[i, (slot, j)] per b --------------------
                    aps = ps3.tile([128, half, nslot * J], F32, tag="aps")
                    for b in range(half):
                        nc.tensor.matmul(
                            aps[:, b, :],
                            u_t[:, b, :],
                            vblk[:, b0 + b, :nslot, :],
                            start=True,
                            stop=True,
                        )
                    av = aps[:].rearrange("p b (s j) -> p b s j", j=J)
                    lg = sm.tile([128, half, J], F32, tag="lg")
                    if it == 2:
                        nc.vector.tensor_copy(lg[:], av[:, :, 0, :])
                    else:
                        lg0 = sm.tile([128, half, J], F32, tag="lg0")
                        nc.scalar.activation(
                            lg0[:], av[:, :, 0, :],
                            mybir.ActivationFunctionType.Copy,
                        )
                        nc.vector.tensor_tensor(
                            lg[:], lg0[:], av[:, :, 1, :], mybir.AluOpType.add
                        )
                    e = sm.tile([128, half, J], BF16, tag="e")
                    nc.scalar.activation(
                        e[:], lg[:], mybir.ActivationFunctionType.Exp
                    )
                    z = sm.tile([128, half], F32, tag="z")
                    nc.vector.tensor_reduce(
                        z[:], e[:], mybir.AxisListType.X, mybir.AluOpType.add
                    )
                    rz = sm.tile([128, half], F32, tag="rz")
                    nc.vector.reciprocal(rz[:], z[:])
                    c_t = sm.tile([128, half, J], BF16, tag="c_t")
                    rzb = bass.AP(
                        rz.tensor, rz[:].offset,
                        [rz[:].ap[0], rz[:].ap[1], [0, J]],
                    )
                    nc.vector.tensor_tensor(
                        c_t[:], e[:], rzb, mybir.AluOpType.mult
                    )
                    if stage < 3:
                        continue
                    # -- s-pass: c stationary, u_res tile moving ------------
                    for b in range(half):
                        nc.tensor.matmul(
                            s_ps[:, b, :],
                            c_t[:, b, :],
                            u_res[:, t, b, :],
                            start=False,
                            stop=False,
                            skip_group_check=True,
                        )
                if stage < 3:
                    vv = vT[b0 : b0 + half, :] if vT.shape[0] > half else vT[:]
                    nc.sync.dma_start(
                        v_out[:].rearrange("b j m -> b (j m)")[b0 : b0 + half, :],
                        vv,
                    )
                    continue
                # -- s extract: mask out j-diagonal, sum the 8 j-rows -------
                msb = sm.tile([J, half, JM], BF16, tag="msb")
                maskT_b = bass.AP(
                    maskT.tensor, maskT[:].offset,
                    [maskT[:].ap[0], [0, half], maskT[:].ap[1]],
                )
                nc.vector.tensor_tensor(
                    msb[:], s_ps[:], maskT_b, mybir.AluOpType.mult
                )
                s2_ps = ps.tile([JM, half], F32, tag="sx")
                for b in range(half):
                    nc.tensor.matmul(
                        s2_ps[:, b : b + 1], msb[:, b, :], ones8[:],
                        start=True, stop=True,
                    )
                s_sb = small.tile([JM, half], F32, tag="s_sb")
                nc.vector.tensor_copy(s_sb[:], s2_ps[:])
                vTh = _squash_chain(nc, small, ps, s_sb, ident, half)
                if it == 2:
                    vb = _vblk_from_vT(nc, small, vTh, mask_rep, half)
                    nc.vector.tensor_copy(vblk[:, b0 : b0 + half, 1, :], vb[:])
                else:
                    nc.sync.dma_start(
                        v_out[:].rearrange("b j m -> b (j m)")[b0 : b0 + half, :],
                        vTh[:],
                    )


_NC_CACHE = {}


def _build_nc(b_c=B_C, half=16, b_blk=2, reps=1, stage=3):
    key = (b_c, half, b_blk, reps, stage)
    if key not in _NC_CACHE:
        _patch_tile()
        nc = bass.Bass("TRN2", target_bir_lowering=False, debug=False)
        wcr_d = nc.dram_tensor("wcr", [128, H, JM], BF16, kind="ExternalInput").ap()
        xblk_d = nc.dram_tensor(
            "xblk", [b_c // half, T, 128, CH_T, half, IP], BF16,
            kind="ExternalInput",
        ).ap()
        xt_d = nc.dram_tensor("xt", [128, H, b_c], BF16, kind="ExternalInput").ap()
        mask_d = nc.dram_tensor("mask", [JM, J], BF16, kind="ExternalInput").ap()
        ident_d = nc.dram_tensor("ident", [128, 128], F32, kind="ExternalInput").ap()
        v_d = nc.dram_tensor("v", [b_c, J, M], F32, kind="ExternalOutput").ap()
        with tile.TileContext(nc) as tc:
            build_kernel(
                tc,
                [v_d],
                [wcr_d, xblk_d, xt_d, mask_d, ident_d],
                b_c=b_c,
                half=half,
                b_blk=b_blk,
                reps=reps,
                stage=stage,
            )
        _split_waits(nc)
        _NC_CACHE[key] = nc
    return _NC_CACHE[key]


def host_prep(x, W):
    """Returns (wcr, xblk_all, xt_all, mask, ident); x-deriveds cover all B.
    Row order of the 128 K-rows is (i16, n): i = h*IP + i16."""
    bf = ml_dtypes.bfloat16
    nb = x.shape[0]
    # wcr[(i16*N + n), h, jm] = W[h*IP + i16, j, n, m]
    Wr = np.ascontiguousarray(W.transpose(0, 2, 1, 3)).reshape(I, N, JM)
    Wr = Wr.reshape(H, IP, N, JM)
    wcr = np.ascontiguousarray(Wr.transpose(1, 2, 0, 3)).reshape(128, H, JM)
    # x rows in the same (i16, n) order per h
    xr = x.reshape(nb, H, IP, N)
    xrows = np.ascontiguousarray(xr.transpose(2, 3, 1, 0)).reshape(128, H, nb)
    rows = np.arange(128)
    i16_of_row = rows // N
    xblk = np.zeros((128, H, nb, IP), np.float32)
    for r in range(128):
        xblk[r, :, :, i16_of_row[r]] = xrows[r]
    mask = np.zeros((JM, J), np.float32)
    for j in range(J):
        mask[j * M : (j + 1) * M, j] = 1.0
    ident = np.eye(128, dtype=np.float32)
    return wcr.astype(bf), xblk.astype(bf), xrows.astype(bf), mask.astype(bf), ident


def regroup(xblk_core, xt_core, half, b_blk=None):
    """xblk [128,H,nb,IP] -> t-major [n_half, T, 128, CH_T, half, IP];
    xt passes through."""
    nb = xblk_core.shape[2]
    n_half = nb // half
    xb = xblk_core.reshape(128, T, CH_T, nb, IP)
    xb = xb.transpose(3, 1, 0, 2, 4)  # [nb, T, 128, CH_T, IP]
    xb = xb.reshape(n_half, half, T, 128, CH_T, IP).transpose(0, 2, 3, 4, 1, 5)
    return np.ascontiguousarray(xb), np.ascontiguousarray(xt_core)


def kernel(x, W):
    x = np.asarray(x, np.float32)
    W = np.asarray(W, np.float32)
    wcr, xblk_all, xt_all, mask, ident = host_prep(x, W)
    nc = _build_nc()
    in_maps = []
    for c in range(N_CORES):
        bs = slice(c * B_C, (c + 1) * B_C)
        xb_c, xt_c = regroup(xblk_all[:, :, bs, :], xt_all[:, :, bs], 16, 2)
        in_maps.append(
            {"wcr": wcr, "xblk": xb_c, "xt": xt_c, "mask": mask, "ident": ident}
        )
    res = run_bass_kernel_spmd(nc, in_maps, list(range(N_CORES)))
    out = np.concatenate([res.results[c]["v"] for c in range(N_CORES)], axis=0)
    return out.astype(np.float32)
